# revision 6
# baseline (speedup 1.0000x reference)
"""Trainium2 Bass kernel for nn_AttentionSelfAttention (B=8, S=1024, D=64, 8 heads).

Sharding: data-parallel over batch — one batch element per NeuronCore (8 cores).
Each core runs the full attention block for its batch element; no collectives.

Per-core layout strategy (everything transposed so the softmax statistics fall
out of the matmuls and the big 32MB attention tensor is written at full DMA
bandwidth):
  xqT, xkT   [64, 1024]        (qx+pos)^T, (kvx+pos)^T                  (f32r)
  qT, kT     [128, 4, 1024]    projected q^T / k^T, two heads per 128 rows
  v_aug      [128, h, jc, 65]  v rows with an extra ones column         (f32r)
  scoresT    [128 j, 1024 i] = k_h @ q_h^T        (PSUM, per 128-row j chunk)
  et         [128, 1024]     = exp(scoresT/8)                           (f32r)
  ctx_ps     [65, 512]       = v_aug^T @ et accumulated over j chunks:
                               rows 0-63 = unnormalized ctx^T, row 64 = softmax
                               row-sum (the ones-column trick)
  B          [128, 1024]     = 1/rowsum broadcast over partitions (PE ones-matmul)
  attnT out  [h, j, i]       = et * B   -> host transposes to attn[h, i, j]
  outT out   [64, 1024]      = sum_h Wo_h^T @ (ctxT_h * B) (+ bo), host transposes

Matmuls run in float32r (single-pass reduced-precision fp32, ~1.2e-4 rel err,
4x faster than native fp32 on the PE).
"""

from contextlib import ExitStack

import numpy as np

import concourse.bacc as bacc
import concourse.mybir as mybir
import concourse.tile as tile
from concourse.bass_utils import run_bass_kernel_spmd
from concourse.masks import make_identity

F32 = mybir.dt.float32
F32R = mybir.dt.float32r
EXP = mybir.ActivationFunctionType.Exp

B = 8
HH = 32
WW = 32
D = 64
NH = 4
S = HH * WW
NHEADS = 2 * NH
JC = S // 128
IC = S // 512
N_CORES = 8

BIAS_NAMES = ("bq", "bka", "bva", "bksa", "bvsa", "bo")
NORM_GP_JCS = (3, 7)  # which attn-normalize j-chunks run on GpSimd instead of DVE


def declare_io(nc, bias_names=()):
    io = {}
    io["qx"] = nc.dram_tensor("qx", [S, D], F32, kind="ExternalInput").ap()
    io["kvx"] = nc.dram_tensor("kvx", [S, D], F32, kind="ExternalInput").ap()
    io["pos"] = nc.dram_tensor("pos", [S, D], F32, kind="ExternalInput").ap()
    io["Wq"] = nc.dram_tensor("Wq", [D, 512], F32, kind="ExternalInput").ap()
    io["Wka"] = nc.dram_tensor("Wka", [D, 256], F32, kind="ExternalInput").ap()
    io["Wva"] = nc.dram_tensor("Wva", [D, 256], F32, kind="ExternalInput").ap()
    io["Wksa"] = nc.dram_tensor("Wksa", [D, 256], F32, kind="ExternalInput").ap()
    io["Wvsa"] = nc.dram_tensor("Wvsa", [D, 256], F32, kind="ExternalInput").ap()
    io["Wo"] = nc.dram_tensor("Wo", [512, D], F32, kind="ExternalInput").ap()
    for b, n in [("bq", 512), ("bka", 256), ("bva", 256),
                 ("bksa", 256), ("bvsa", 256), ("bo", 64)]:
        if b in bias_names:
            io[b] = nc.dram_tensor(b, [1, n], F32, kind="ExternalInput").ap()
    io["attn"] = nc.dram_tensor("attn", [NHEADS, S, S], F32, kind="ExternalOutput").ap()
    io["outT"] = nc.dram_tensor("outT", [D, S], F32, kind="ExternalOutput").ap()
    return io


def build_attention(ctx: ExitStack, tc, io, bias_names=()):
    nc = tc.nc
    has = lambda b: b in bias_names

    const = ctx.enter_context(tc.tile_pool(name="const", bufs=1))
    wpool = ctx.enter_context(tc.tile_pool(name="weights", bufs=1))
    xpool = ctx.enter_context(tc.tile_pool(name="x", bufs=1))
    qkpool = ctx.enter_context(tc.tile_pool(name="qk", bufs=1))
    etpool = ctx.enter_context(tc.tile_pool(name="et", bufs=9 if bias_names else 10))
    atpool = ctx.enter_context(tc.tile_pool(name="at", bufs=3 if bias_names else 4))
    bpool = ctx.enter_context(tc.tile_pool(name="bcast", bufs=2))
    spool = ctx.enter_context(tc.tile_pool(name="small", bufs=2))
    ctxpool = ctx.enter_context(tc.tile_pool(name="ctxTn", bufs=1))

    ps_s_pool = ctx.enter_context(tc.tile_pool(name="ps_s", bufs=2, space="PSUM"))
    ps_ctx_pool = ctx.enter_context(tc.tile_pool(name="ps_ctx", bufs=3, space="PSUM"))
    ps_misc_pool = ctx.enter_context(tc.tile_pool(name="ps_misc", bufs=1, space="PSUM"))

    # ---- constants ----
    identity = const.tile([128, 128], F32)
    make_identity(nc, identity[:])
    ones_f = const.tile([1, 512], F32)
    nc.vector.memset(ones_f[:], 1.0)
    ones_r = const.tile([1, 512], F32R)
    nc.vector.tensor_copy(ones_r[:], ones_f[:])
    ones_col = const.tile([128, 2], F32)
    nc.vector.memset(ones_col[:], 1.0)

    BF16 = mybir.dt.bfloat16
    warm_w = const.tile([128, 128], BF16)
    nc.vector.memset(warm_w[:], 1.0)
    warm_x = const.tile([128, 512], BF16)
    nc.vector.memset(warm_x[:], 1.0)
    ps_w = ps_misc_pool.tile([128, 512], F32, tag="mm", name="ps_warm")
    for i in range(16):
        nc.tensor.matmul(ps_w[:], warm_w[:], warm_x[:], start=True, stop=True)

    # ---- weights (DMA with f32->f32r rounding on gpsimd) ----
    wq_t = wpool.tile([D, 512], F32R)
    nc.gpsimd.dma_start(wq_t[:], io["Wq"])
    wka_t = wpool.tile([D, 256], F32R)
    nc.gpsimd.dma_start(wka_t[:], io["Wka"])
    wksa_t = wpool.tile([D, 256], F32R)
    nc.gpsimd.dma_start(wksa_t[:], io["Wksa"])
    wva_t = wpool.tile([D, 256], F32R)
    nc.gpsimd.dma_start(wva_t[:], io["Wva"])
    wvsa_t = wpool.tile([D, 256], F32R)
    nc.gpsimd.dma_start(wvsa_t[:], io["Wvsa"])
    wo_t = wpool.tile([D, NHEADS, D], F32R)
    nc.gpsimd.dma_start(wo_t[:], io["Wo"].rearrange("(h d) e -> d h e", d=D))

    bias_sb = {}
    for b in bias_names:
        n = io[b].shape[1]
        t = wpool.tile([1, n], F32R, name=f"b_{b}")
        nc.gpsimd.dma_start(t[:], io[b])
        bias_sb[b] = t

    def bias_bcast(b, sl, rows, cols):
        """SBUF tile [rows, cols] = bias_sb[b][0, sl] per-partition, bcast over free."""
        ps = ps_misc_pool.tile([rows, cols], F32, tag="mm", name=f"ps_bb_{b}_{sl.start}")
        nc.tensor.matmul(ps[:], bias_sb[b][:, sl], ones_r[:, 0:cols],
                         start=True, stop=True)
        t = wpool.tile([rows, cols], F32, name=f"bb_{b}_{sl.start}")
        nc.scalar.copy(t[:], ps[:])
        return t

    def bias_bcast_free(b, rows):
        """SBUF tile [rows, n] = bias_sb[b] per-free-element, bcast over partitions."""
        n = io[b].shape[1]
        ps = ps_misc_pool.tile([rows, n], F32, tag="mm", name=f"ps_bf_{b}")
        nc.tensor.matmul(ps[:], ones_r[:, 0:rows], bias_sb[b][:],
                         start=True, stop=True)
        t = wpool.tile([rows, n], F32, name=f"bf_{b}")
        nc.scalar.copy(t[:], ps[:])
        return t

    bb_v = {}
    if has("bva"):
        bb_v["bva"] = bias_bcast_free("bva", 128)
    if has("bvsa"):
        bb_v["bvsa"] = bias_bcast_free("bvsa", 128)
    bb_q = [bias_bcast("bq", slice(c * 128, (c + 1) * 128), 128, 512)
            for c in range(4)] if has("bq") else None
    bb_k = []
    for c in range(4):
        bk = "bka" if c < 2 else "bksa"
        cc = c % 2
        bb_k.append(bias_bcast(bk, slice(cc * 128, (cc + 1) * 128), 128, 512)
                    if has(bk) else None)
    bb_o = bias_bcast("bo", slice(0, 64), 64, 512) if has("bo") else None

    # ---- inputs + positional add ----
    xq = xpool.tile([128, JC, D], F32)
    nc.sync.dma_start(xq[:], io["qx"].rearrange("(o p) d -> p o d", p=128))
    xk = xpool.tile([128, JC, D], F32)
    nc.sync.dma_start(xk[:], io["kvx"].rearrange("(o p) d -> p o d", p=128))
    pos_t = xpool.tile([128, JC, D], F32)
    nc.sync.dma_start(pos_t[:], io["pos"].rearrange("(o p) d -> p o d", p=128))
    nc.vector.tensor_add(xq[:], xq[:], pos_t[:])
    nc.vector.tensor_add(xk[:], xk[:], pos_t[:])

    # ---- transpose inputs: xT[d, i] (f32r) ----
    xqT = xpool.tile([D, S], F32R)
    xkT = xpool.tile([D, S], F32R)
    for xt, x in ((xqT, xq), (xkT, xk)):
        for oc in range(JC):
            ps_t = ps_misc_pool.tile([D, 128], F32, tag="mm")
            nc.tensor.transpose(ps_t[:], x[:, oc, :], identity[:])
            nc.vector.tensor_copy(xt[:, oc * 128:(oc + 1) * 128], ps_t[:])

    # ---- v projection (+ ones column) ----
    # v_aug[p, h, jc, 0:64] = (x_p @ Wv + bv)[jc*128+p, h*64:...]; [..., 64] = 1
    v_aug = xpool.tile([128, NHEADS, JC, 65], F32R)
    for g, (xt, wv, bv) in enumerate(
        ((xkT, wva_t, "bva"), (xqT, wvsa_t, "bvsa"))
    ):
        for jc in range(JC):
            ps_v = ps_misc_pool.tile([128, 256], F32, tag="mm")
            nc.tensor.matmul(ps_v[:], xt[:, jc * 128:(jc + 1) * 128], wv[:],
                             start=True, stop=True)
            dst = v_aug[:, g * 4:(g + 1) * 4, jc, 0:D]
            if has(bv):
                nc.vector.tensor_add(
                    dst, ps_v[:].rearrange("p (h d) -> p h d", h=4),
                    bb_v[bv][:].rearrange("p (h d) -> p h d", h=4))
            else:
                nc.vector.tensor_copy(dst, ps_v[:].rearrange("p (h d) -> p h d", h=4))
    nc.vector.tensor_copy(
        v_aug[:, :, :, D:D + 1],
        ones_col[:, 0:1].unsqueeze(1).to_broadcast([128, NHEADS, JC, 1]))

    # ---- q/k head projections, 2 heads (128 rows) per matmul ----
    # qT_all[p, c, i]: rows p = (h%2)*64+d for head h = 2c + p//64
    qT_all = qkpool.tile([128, 4, S], F32R, tag="qT")
    for ic in range(IC):
        for c in range(4):
            ps_q = ps_misc_pool.tile([128, 512], F32, tag="mm")
            nc.tensor.matmul(ps_q[:], wq_t[:, c * 128:(c + 1) * 128],
                             xqT[:, ic * 512:(ic + 1) * 512],
                             start=True, stop=True)
            dst = qT_all[:, c, ic * 512:(ic + 1) * 512]
            if has("bq"):
                nc.vector.tensor_add(dst, ps_q[:], bb_q[c][:])
            else:
                nc.vector.tensor_copy(dst, ps_q[:])

    # kT: heads 0-3 = Wka on xkT; heads 4-7 = Wksa on xqT
    kT_all = qkpool.tile([128, 4, S], F32R, tag="kT")
    for ic in range(IC):
        for c in range(4):
            xt = xkT if c < 2 else xqT
            wk = wka_t if c < 2 else wksa_t
            bk = "bka" if c < 2 else "bksa"
            cc = c % 2
            ps_k = ps_misc_pool.tile([128, 512], F32, tag="mm")
            nc.tensor.matmul(ps_k[:], wk[:, cc * 128:(cc + 1) * 128],
                             xt[:, ic * 512:(ic + 1) * 512],
                             start=True, stop=True)
            dst = kT_all[:, c, ic * 512:(ic + 1) * 512]
            if has(bk):
                nc.vector.tensor_add(dst, ps_k[:], bb_k[c][:])
            else:
                nc.vector.tensor_copy(dst, ps_k[:])

    def qT_h(h):
        return qT_all[(h % 2) * 64:(h % 2) * 64 + 64, h // 2, :]

    def kT_h(h):
        return kT_all[(h % 2) * 64:(h % 2) * 64 + 64, h // 2, :]

    # ---- per-head attention ----
    # kdup/vdup break the fp32r same-stationary-operand hazard at the two
    # points where a pair of matmuls would otherwise share an lhsT address.
    kdup = []
    vdup = []
    for h in range(NHEADS):
        kd = spool.tile([128, 128], F32R, tag="kdup", name=f"kdup{h}")
        p0 = (h % 2) * 64
        nc.vector.tensor_copy(kd[p0:p0 + 64, :], kT_h(h)[:, 0:128])
        kdup.append(kd[p0:p0 + 64, :])
        vd = spool.tile([128, 65], F32R, tag="vdup", name=f"vdup{h}")
        nc.vector.tensor_copy(vd[:], v_aug[:, h, JC - 1, :])
        vdup.append(vd)

    ctxTn = ctxpool.tile([D, NHEADS, S], F32R)
    for h in range(NHEADS):
        ps_c = [ps_ctx_pool.tile([65, 512], F32, tag="ctx", name=f"ps_c{h}_{i}")
                for i in range(IC)]
        ets = []
        pend = None
        for jc in range(JC):
            ps_s = ps_s_pool.tile([128, S], F32, tag="s")
            # software pipeline: scores(jc) interleaved with ctx(jc-1) so no
            # two consecutive PE matmuls share a stationary operand.
            kT_ic1 = kdup[h] if jc == 0 else kT_h(h)[:, jc * 128:(jc + 1) * 128]
            nc.tensor.matmul(ps_s[:, 0:512],
                             kT_h(h)[:, jc * 128:(jc + 1) * 128],
                             qT_h(h)[:, 0:512], start=True, stop=True)
            if pend is not None:
                pet, pjc = pend
                nc.tensor.matmul(ps_c[0][:], v_aug[:, h, pjc, :],
                                 pet[:, 0:512], start=(pjc == 0), stop=False)
            nc.tensor.matmul(ps_s[:, 512:1024], kT_ic1,
                             qT_h(h)[:, 512:1024], start=True, stop=True)
            if pend is not None:
                pet, pjc = pend
                nc.tensor.matmul(ps_c[1][:], v_aug[:, h, pjc, :],
                                 pet[:, 512:1024], start=(pjc == 0), stop=False)
            et = etpool.tile([128, S], F32R, tag="et")
            nc.scalar.activation(et[:], ps_s[:], EXP, scale=0.125)
            ets.append(et)
            pend = (et, jc)
        # epilogue ctx pair (vdup on the second to avoid the lhsT repeat)
        nc.tensor.matmul(ps_c[0][:], v_aug[:, h, JC - 1, :],
                         ets[JC - 1][:, 0:512], start=False, stop=True)
        nc.tensor.matmul(ps_c[1][:], vdup[h][:],
                         ets[JC - 1][:, 512:1024], start=False, stop=True)

        # 1/rowsum: spread the 1024 sums over 128 partitions by DMA (cheap
        # elementwise reciprocal), gather back, broadcast over partitions.
        sums = spool.tile([1, S], F32, tag="sums")
        nc.scalar.copy(sums[:, 0:512], ps_c[0][64:65, :])
        nc.scalar.copy(sums[:, 512:1024], ps_c[1][64:65, :])
        sp = spool.tile([128, 8], F32, tag="sp")
        nc.sync.dma_start(sp[:], sums[:])
        nc.vector.reciprocal(sp[:], sp[:])
        rrow = spool.tile([1, S], F32, tag="rrow")
        nc.sync.dma_start(rrow[:], sp[:])
        B_t = bpool.tile([128, S], F32, tag="B")
        nc.gpsimd.partition_broadcast(B_t[:, 0:512], rrow[:, 0:512])
        nc.gpsimd.partition_broadcast(B_t[:, 512:1024], rrow[:, 512:1024])
        for ic in range(IC):
            nc.vector.tensor_mul(ctxTn[:, h, ic * 512:(ic + 1) * 512],
                                 ps_c[ic][0:D, :],
                                 B_t[0:D, ic * 512:(ic + 1) * 512])
        # normalize attn^T and write out (NORM_GP_JCS tiles go to GpSimd)
        for jc in range(JC):
            at = atpool.tile([128, S], F32, tag="at")
            eng = nc.gpsimd if jc in NORM_GP_JCS else nc.vector
            eng.tensor_mul(at[:], ets[jc][:].bitcast(F32), B_t[:])
            nc.sync.dma_start(io["attn"][h, jc * 128:(jc + 1) * 128, :], at[:])

    # ---- output projection: outT = sum_h Wo_h^T @ ctxTn_h (+ bo) ----
    for ic in range(IC):
        ps_o = ps_misc_pool.tile([D, 512], F32, tag="mm")
        for h in range(NHEADS):
            nc.tensor.matmul(ps_o[:], wo_t[:, h, :],
                             ctxTn[:, h, ic * 512:(ic + 1) * 512],
                             start=(h == 0), stop=(h == NHEADS - 1))
        ot = spool.tile([D, 512], F32, tag="ot")
        if has("bo"):
            nc.vector.tensor_add(ot[:], ps_o[:], bb_o[:])
        else:
            nc.vector.tensor_copy(ot[:], ps_o[:])
        nc.sync.dma_start(io["outT"][:, ic * 512:(ic + 1) * 512], ot[:])


_nc_cache = {}


def _get_nc(bias_names):
    key = tuple(bias_names)
    if key not in _nc_cache:
        nc = bacc.Bacc()
        io = declare_io(nc, bias_names)
        with tile.TileContext(nc) as tc:
            with ExitStack() as ctx:
                build_attention(ctx, tc, io, bias_names)
        nc.finalize()
        _nc_cache[key] = nc
    return _nc_cache[key]


def _run(inputs, trace=False, **run_kwargs):
    qx = np.ascontiguousarray(np.asarray(inputs["qx"], dtype=np.float32).reshape(B, S, D))
    kvx = np.ascontiguousarray(np.asarray(inputs["kvx"], dtype=np.float32).reshape(B, S, D))
    pos = np.ascontiguousarray(np.asarray(inputs["pos_table"], dtype=np.float32))
    common = {
        "pos": pos,
        "Wq": np.ascontiguousarray(np.asarray(inputs["Wq"], dtype=np.float32)),
        "Wka": np.ascontiguousarray(np.asarray(inputs["Wka"], dtype=np.float32)),
        "Wva": np.ascontiguousarray(np.asarray(inputs["Wva"], dtype=np.float32)),
        "Wksa": np.ascontiguousarray(np.asarray(inputs["Wksa"], dtype=np.float32)),
        "Wvsa": np.ascontiguousarray(np.asarray(inputs["Wvsa"], dtype=np.float32)),
        "Wo": np.ascontiguousarray(np.asarray(inputs["Wo"], dtype=np.float32)),
    }
    bias_names = tuple(b for b in BIAS_NAMES if np.any(np.asarray(inputs[b])))
    for b in bias_names:
        common[b] = np.ascontiguousarray(
            np.asarray(inputs[b], dtype=np.float32).reshape(1, -1))

    nc = _get_nc(bias_names)
    in_maps = [{"qx": qx[c], "kvx": kvx[c], **common} for c in range(N_CORES)]
    res = run_bass_kernel_spmd(nc, in_maps, core_ids=list(range(N_CORES)),
                               trace=trace, **run_kwargs)

    out = np.empty((B, HH, WW, D), np.float32)
    attn = np.empty((B, NHEADS, S, S), np.float32)
    for c in range(N_CORES):
        r = res.results[c]
        attn[c] = r["attn"].transpose(0, 2, 1)
        out[c] = r["outT"].T.reshape(HH, WW, D)
    return (out, attn), res


def kernel(**inputs):
    (out, attn), _ = _run(inputs)
    return out, attn


# revision 7
# speedup vs baseline: 1.2918x; 1.2918x over previous
"""Trainium2 Bass kernel for nn_AttentionSelfAttention (B=8, S=1024, D=64, 8 heads).

Sharding: data-parallel over batch — one batch element per NeuronCore (8 cores).
Each core runs the full attention block for its batch element; no collectives.

Per-core layout strategy (everything transposed so the softmax statistics fall
out of the matmuls and the big 32MB attention tensor is written at full DMA
bandwidth):
  xqT, xkT   [64, 1024]        (qx+pos)^T, (kvx+pos)^T                  (f32r)
  qT, kT     [128, 4, 1024]    projected q^T / k^T, two heads per 128 rows
  v_aug      [128, h, jc, 65]  v rows with an extra ones column         (f32r)
  scoresT    [128 j, 1024 i] = k_h @ q_h^T        (PSUM, per 128-row j chunk)
  et         [128, 1024]     = exp(scoresT/8)                           (f32r)
  ctx_ps     [65, 512]       = v_aug^T @ et accumulated over j chunks:
                               rows 0-63 = unnormalized ctx^T, row 64 = softmax
                               row-sum (the ones-column trick)
  B          [128, 1024]     = 1/rowsum broadcast over partitions (PE ones-matmul)
  attnT out  [h, j, i]       = et * B   -> host transposes to attn[h, i, j]
  outT out   [64, 1024]      = sum_h Wo_h^T @ (ctxT_h * B) (+ bo), host transposes

Matmuls run in float32r (single-pass reduced-precision fp32, ~1.2e-4 rel err,
4x faster than native fp32 on the PE).
"""

from contextlib import ExitStack

import numpy as np

import concourse.bacc as bacc
import concourse.mybir as mybir
import concourse.tile as tile
from concourse.bass_utils import run_bass_kernel_spmd
from concourse.masks import make_identity

F32 = mybir.dt.float32
F32R = mybir.dt.float32r
EXP = mybir.ActivationFunctionType.Exp

B = 8
HH = 32
WW = 32
D = 64
NH = 4
S = HH * WW
NHEADS = 2 * NH
JC = S // 128
IC = S // 512
N_CORES = 8

BIAS_NAMES = ("bq", "bka", "bva", "bksa", "bvsa", "bo")
NORM_GP_JCS = ()  # which attn-normalize j-chunks run on GpSimd instead of DVE


def declare_io(nc, bias_names=()):
    io = {}
    io["qx"] = nc.dram_tensor("qx", [S, D], F32, kind="ExternalInput").ap()
    io["kvx"] = nc.dram_tensor("kvx", [S, D], F32, kind="ExternalInput").ap()
    io["pos"] = nc.dram_tensor("pos", [S, D], F32, kind="ExternalInput").ap()
    io["Wq"] = nc.dram_tensor("Wq", [D, 512], F32, kind="ExternalInput").ap()
    io["Wka"] = nc.dram_tensor("Wka", [D, 256], F32, kind="ExternalInput").ap()
    io["Wva"] = nc.dram_tensor("Wva", [D, 256], F32, kind="ExternalInput").ap()
    io["Wksa"] = nc.dram_tensor("Wksa", [D, 256], F32, kind="ExternalInput").ap()
    io["Wvsa"] = nc.dram_tensor("Wvsa", [D, 256], F32, kind="ExternalInput").ap()
    io["Wo"] = nc.dram_tensor("Wo", [512, D], F32, kind="ExternalInput").ap()
    for b, n in [("bq", 512), ("bka", 256), ("bva", 256),
                 ("bksa", 256), ("bvsa", 256), ("bo", 64)]:
        if b in bias_names:
            io[b] = nc.dram_tensor(b, [1, n], F32, kind="ExternalInput").ap()
    io["attn"] = nc.dram_tensor("attn", [NHEADS, S, S], F32, kind="ExternalOutput").ap()
    io["outT"] = nc.dram_tensor("outT", [D, S], F32, kind="ExternalOutput").ap()
    return io


def build_attention(ctx: ExitStack, tc, io, bias_names=()):
    nc = tc.nc
    has = lambda b: b in bias_names

    const = ctx.enter_context(tc.tile_pool(name="const", bufs=1))
    wpool = ctx.enter_context(tc.tile_pool(name="weights", bufs=1))
    xpool = ctx.enter_context(tc.tile_pool(name="x", bufs=1))
    qkpool = ctx.enter_context(tc.tile_pool(name="qk", bufs=1))
    etpool = ctx.enter_context(tc.tile_pool(name="et", bufs=9 if bias_names else 10))
    atpool = ctx.enter_context(tc.tile_pool(name="at", bufs=3 if bias_names else 4))
    bpool = ctx.enter_context(tc.tile_pool(name="bcast", bufs=2))
    spool = ctx.enter_context(tc.tile_pool(name="small", bufs=2))
    ctxpool = ctx.enter_context(tc.tile_pool(name="ctxTn", bufs=1))

    ps_s_pool = ctx.enter_context(tc.tile_pool(name="ps_s", bufs=2, space="PSUM"))
    ps_ctx_pool = ctx.enter_context(tc.tile_pool(name="ps_ctx", bufs=2, space="PSUM"))
    ps_misc_pool = ctx.enter_context(tc.tile_pool(name="ps_misc", bufs=2, space="PSUM"))

    # ---- constants ----
    identity = const.tile([128, 128], F32)
    make_identity(nc, identity[:])
    ones_f = const.tile([1, 512], F32)
    nc.vector.memset(ones_f[:], 1.0)
    ones_r = const.tile([1, 512], F32R)
    nc.vector.tensor_copy(ones_r[:], ones_f[:])
    ones_col = const.tile([128, 2], F32)
    nc.vector.memset(ones_col[:], 1.0)

    BF16 = mybir.dt.bfloat16
    warm_w = const.tile([128, 128], BF16)
    nc.vector.memset(warm_w[:], 1.0)
    warm_x = const.tile([128, 512], BF16)
    nc.vector.memset(warm_x[:], 1.0)
    ps_w = ps_misc_pool.tile([128, 512], F32, tag="mm", name="ps_warm")
    for i in range(16):
        nc.tensor.matmul(ps_w[:], warm_w[:], warm_x[:], start=True, stop=True)

    # ---- weights (DMA with f32->f32r rounding on gpsimd) ----
    wq_t = wpool.tile([D, 512], F32R)
    nc.gpsimd.dma_start(wq_t[:], io["Wq"])
    wka_t = wpool.tile([D, 256], F32R)
    nc.gpsimd.dma_start(wka_t[:], io["Wka"])
    wksa_t = wpool.tile([D, 256], F32R)
    nc.gpsimd.dma_start(wksa_t[:], io["Wksa"])
    wva_t = wpool.tile([D, 256], F32R)
    nc.gpsimd.dma_start(wva_t[:], io["Wva"])
    wvsa_t = wpool.tile([D, 256], F32R)
    nc.gpsimd.dma_start(wvsa_t[:], io["Wvsa"])
    wo_t = wpool.tile([D, NHEADS, D], F32R)
    nc.gpsimd.dma_start(wo_t[:], io["Wo"].rearrange("(h d) e -> d h e", d=D))

    bias_sb = {}
    for b in bias_names:
        n = io[b].shape[1]
        t = wpool.tile([1, n], F32R, name=f"b_{b}")
        nc.gpsimd.dma_start(t[:], io[b])
        bias_sb[b] = t

    def bias_bcast(b, sl, rows, cols):
        """SBUF tile [rows, cols] = bias_sb[b][0, sl] per-partition, bcast over free."""
        ps = ps_misc_pool.tile([rows, cols], F32, tag="mm", name=f"ps_bb_{b}_{sl.start}")
        nc.tensor.matmul(ps[:], bias_sb[b][:, sl], ones_r[:, 0:cols],
                         start=True, stop=True)
        t = wpool.tile([rows, cols], F32, name=f"bb_{b}_{sl.start}")
        nc.scalar.copy(t[:], ps[:])
        return t

    def bias_bcast_free(b, rows):
        """SBUF tile [rows, n] = bias_sb[b] per-free-element, bcast over partitions."""
        n = io[b].shape[1]
        ps = ps_misc_pool.tile([rows, n], F32, tag="mm", name=f"ps_bf_{b}")
        nc.tensor.matmul(ps[:], ones_r[:, 0:rows], bias_sb[b][:],
                         start=True, stop=True)
        t = wpool.tile([rows, n], F32, name=f"bf_{b}")
        nc.scalar.copy(t[:], ps[:])
        return t

    bb_v = {}
    if has("bva"):
        bb_v["bva"] = bias_bcast_free("bva", 128)
    if has("bvsa"):
        bb_v["bvsa"] = bias_bcast_free("bvsa", 128)
    bb_q = [bias_bcast("bq", slice(c * 128, (c + 1) * 128), 128, 512)
            for c in range(4)] if has("bq") else None
    bb_k = []
    for c in range(4):
        bk = "bka" if c < 2 else "bksa"
        cc = c % 2
        bb_k.append(bias_bcast(bk, slice(cc * 128, (cc + 1) * 128), 128, 512)
                    if has(bk) else None)
    bb_o = bias_bcast("bo", slice(0, 64), 64, 512) if has("bo") else None

    # ---- inputs + positional add ----
    xq = xpool.tile([128, JC, D], F32)
    nc.sync.dma_start(xq[:], io["qx"].rearrange("(o p) d -> p o d", p=128))
    xk = xpool.tile([128, JC, D], F32)
    nc.sync.dma_start(xk[:], io["kvx"].rearrange("(o p) d -> p o d", p=128))
    pos_t = xpool.tile([128, JC, D], F32)
    nc.sync.dma_start(pos_t[:], io["pos"].rearrange("(o p) d -> p o d", p=128))
    nc.vector.tensor_add(xq[:], xq[:], pos_t[:])
    nc.vector.tensor_add(xk[:], xk[:], pos_t[:])

    # ---- transpose inputs: xT[d, i] (f32r) ----
    xqT = xpool.tile([D, S], F32R)
    xkT = xpool.tile([D, S], F32R)
    for xt, x in ((xqT, xq), (xkT, xk)):
        for oc in range(JC):
            ps_t = ps_misc_pool.tile([D, 128], F32, tag="mm")
            nc.tensor.transpose(ps_t[:], x[:, oc, :], identity[:])
            nc.vector.tensor_copy(xt[:, oc * 128:(oc + 1) * 128], ps_t[:])

    # ---- v projection (+ ones column) ----
    # v_aug[p, h, jc, 0:64] = (x_p @ Wv + bv)[jc*128+p, h*64:...]; [..., 64] = 1
    v_aug = xpool.tile([128, NHEADS, JC, 65], F32R)
    for g, (xt, wv, bv) in enumerate(
        ((xkT, wva_t, "bva"), (xqT, wvsa_t, "bvsa"))
    ):
        for jc in range(JC):
            ps_v = ps_misc_pool.tile([128, 256], F32, tag="mm")
            nc.tensor.matmul(ps_v[:], xt[:, jc * 128:(jc + 1) * 128], wv[:],
                             start=True, stop=True)
            dst = v_aug[:, g * 4:(g + 1) * 4, jc, 0:D]
            if has(bv):
                nc.vector.tensor_add(
                    dst, ps_v[:].rearrange("p (h d) -> p h d", h=4),
                    bb_v[bv][:].rearrange("p (h d) -> p h d", h=4))
            else:
                nc.vector.tensor_copy(dst, ps_v[:].rearrange("p (h d) -> p h d", h=4))
    nc.vector.tensor_copy(
        v_aug[:, :, :, D:D + 1],
        ones_col[:, 0:1].unsqueeze(1).to_broadcast([128, NHEADS, JC, 1]))

    # ---- q/k head projections, 2 heads (128 rows) per matmul ----
    # qT_all[p, c, i]: rows p = (h%2)*64+d for head h = 2c + p//64
    qT_all = qkpool.tile([128, 4, S], F32R, tag="qT")
    for ic in range(IC):
        for c in range(4):
            ps_q = ps_misc_pool.tile([128, 512], F32, tag="mm")
            nc.tensor.matmul(ps_q[:], wq_t[:, c * 128:(c + 1) * 128],
                             xqT[:, ic * 512:(ic + 1) * 512],
                             start=True, stop=True)
            dst = qT_all[:, c, ic * 512:(ic + 1) * 512]
            if has("bq"):
                nc.vector.tensor_add(dst, ps_q[:], bb_q[c][:])
            else:
                nc.vector.tensor_copy(dst, ps_q[:])

    # kT: heads 0-3 = Wka on xkT; heads 4-7 = Wksa on xqT
    kT_all = qkpool.tile([128, 4, S], F32R, tag="kT")
    for ic in range(IC):
        for c in range(4):
            xt = xkT if c < 2 else xqT
            wk = wka_t if c < 2 else wksa_t
            bk = "bka" if c < 2 else "bksa"
            cc = c % 2
            ps_k = ps_misc_pool.tile([128, 512], F32, tag="mm")
            nc.tensor.matmul(ps_k[:], wk[:, cc * 128:(cc + 1) * 128],
                             xt[:, ic * 512:(ic + 1) * 512],
                             start=True, stop=True)
            dst = kT_all[:, c, ic * 512:(ic + 1) * 512]
            if has(bk):
                nc.vector.tensor_add(dst, ps_k[:], bb_k[c][:])
            else:
                nc.vector.tensor_copy(dst, ps_k[:])

    def qT_h(h):
        return qT_all[(h % 2) * 64:(h % 2) * 64 + 64, h // 2, :]

    def kT_h(h):
        return kT_all[(h % 2) * 64:(h % 2) * 64 + 64, h // 2, :]

    # ---- per-head attention ----
    # kdup/vdup break the fp32r same-stationary-operand hazard at the two
    # points where a pair of matmuls would otherwise share an lhsT address.
    kdup = []
    vdup = []
    for h in range(NHEADS):
        kd = spool.tile([128, 128], F32R, tag="kdup", name=f"kdup{h}")
        p0 = (h % 2) * 64
        nc.vector.tensor_copy(kd[p0:p0 + 64, :], kT_h(h)[:, 0:128])
        kdup.append(kd[p0:p0 + 64, :])
        vd = spool.tile([128, 65], F32R, tag="vdup", name=f"vdup{h}")
        nc.vector.tensor_copy(vd[:], v_aug[:, h, JC - 1, :])
        vdup.append(vd)

    ps_w2 = ps_s_pool.tile([128, S], F32, tag="s", name="ps_warm2")
    for i in range(8):
        nc.tensor.matmul(ps_w2[:, 0:512], warm_w[:], warm_x[:], start=True, stop=True)

    ctxTn = ctxpool.tile([D, NHEADS, S], F32R)
    for h in range(NHEADS):
        ps_c = [ps_ctx_pool.tile([65, 512], F32, tag="ctx", name=f"ps_c{h}_{i}")
                for i in range(IC)]
        ets = []
        pend = None
        for jc in range(JC):
            ps_s = ps_s_pool.tile([128, S], F32, tag="s")
            # software pipeline: scores(jc) interleaved with ctx(jc-1) so no
            # two consecutive PE matmuls share a stationary operand.
            kT_ic1 = kdup[h] if jc == 0 else kT_h(h)[:, jc * 128:(jc + 1) * 128]
            nc.tensor.matmul(ps_s[:, 0:512],
                             kT_h(h)[:, jc * 128:(jc + 1) * 128],
                             qT_h(h)[:, 0:512], start=True, stop=True)
            if pend is not None:
                pet, pjc = pend
                nc.tensor.matmul(ps_c[0][:], v_aug[:, h, pjc, :],
                                 pet[:, 0:512], start=(pjc == 0), stop=False)
            nc.tensor.matmul(ps_s[:, 512:1024], kT_ic1,
                             qT_h(h)[:, 512:1024], start=True, stop=True)
            if pend is not None:
                pet, pjc = pend
                nc.tensor.matmul(ps_c[1][:], v_aug[:, h, pjc, :],
                                 pet[:, 512:1024], start=(pjc == 0), stop=False)
            et = etpool.tile([128, S], F32R, tag="et")
            nc.scalar.activation(et[:], ps_s[:], EXP, scale=0.125)
            ets.append(et)
            pend = (et, jc)
        # epilogue ctx pair (vdup on the second to avoid the lhsT repeat)
        nc.tensor.matmul(ps_c[0][:], v_aug[:, h, JC - 1, :],
                         ets[JC - 1][:, 0:512], start=False, stop=True)
        nc.tensor.matmul(ps_c[1][:], vdup[h][:],
                         ets[JC - 1][:, 512:1024], start=False, stop=True)

        # 1/rowsum: spread the 1024 sums over 128 partitions by DMA (cheap
        # elementwise reciprocal), gather back, broadcast over partitions.
        sums = spool.tile([1, S], F32, tag="sums")
        nc.scalar.copy(sums[:, 0:512], ps_c[0][64:65, :])
        nc.scalar.copy(sums[:, 512:1024], ps_c[1][64:65, :])
        sp = spool.tile([128, 8], F32, tag="sp")
        nc.sync.dma_start(sp[:], sums[:])
        nc.vector.reciprocal(sp[:], sp[:])
        rrow = spool.tile([1, S], F32, tag="rrow")
        nc.sync.dma_start(rrow[:], sp[:])
        B_t = bpool.tile([128, S], F32, tag="B")
        nc.gpsimd.partition_broadcast(B_t[:, 0:512], rrow[:, 0:512])
        nc.gpsimd.partition_broadcast(B_t[:, 512:1024], rrow[:, 512:1024])
        for ic in range(IC):
            nc.vector.tensor_mul(ctxTn[:, h, ic * 512:(ic + 1) * 512],
                                 ps_c[ic][0:D, :],
                                 B_t[0:D, ic * 512:(ic + 1) * 512])
        # normalize attn^T and write out (NORM_GP_JCS tiles go to GpSimd)
        for jc in range(JC):
            at = atpool.tile([128, S], F32, tag="at")
            eng = nc.gpsimd if jc in NORM_GP_JCS else nc.vector
            eng.tensor_mul(at[:], ets[jc][:].bitcast(F32), B_t[:])
            nc.sync.dma_start(io["attn"][h, jc * 128:(jc + 1) * 128, :], at[:])

    # ---- output projection: outT = sum_h Wo_h^T @ ctxTn_h (+ bo) ----
    for ic in range(IC):
        ps_o = ps_misc_pool.tile([D, 512], F32, tag="mm")
        for h in range(NHEADS):
            nc.tensor.matmul(ps_o[:], wo_t[:, h, :],
                             ctxTn[:, h, ic * 512:(ic + 1) * 512],
                             start=(h == 0), stop=(h == NHEADS - 1))
        ot = spool.tile([D, 512], F32, tag="ot")
        if has("bo"):
            nc.vector.tensor_add(ot[:], ps_o[:], bb_o[:])
        else:
            nc.vector.tensor_copy(ot[:], ps_o[:])
        nc.sync.dma_start(io["outT"][:, ic * 512:(ic + 1) * 512], ot[:])


_nc_cache = {}


def _get_nc(bias_names):
    key = tuple(bias_names)
    if key not in _nc_cache:
        nc = bacc.Bacc()
        io = declare_io(nc, bias_names)
        with tile.TileContext(nc) as tc:
            with ExitStack() as ctx:
                build_attention(ctx, tc, io, bias_names)
        nc.finalize()
        _nc_cache[key] = nc
    return _nc_cache[key]


def _run(inputs, trace=False, **run_kwargs):
    qx = np.ascontiguousarray(np.asarray(inputs["qx"], dtype=np.float32).reshape(B, S, D))
    kvx = np.ascontiguousarray(np.asarray(inputs["kvx"], dtype=np.float32).reshape(B, S, D))
    pos = np.ascontiguousarray(np.asarray(inputs["pos_table"], dtype=np.float32))
    common = {
        "pos": pos,
        "Wq": np.ascontiguousarray(np.asarray(inputs["Wq"], dtype=np.float32)),
        "Wka": np.ascontiguousarray(np.asarray(inputs["Wka"], dtype=np.float32)),
        "Wva": np.ascontiguousarray(np.asarray(inputs["Wva"], dtype=np.float32)),
        "Wksa": np.ascontiguousarray(np.asarray(inputs["Wksa"], dtype=np.float32)),
        "Wvsa": np.ascontiguousarray(np.asarray(inputs["Wvsa"], dtype=np.float32)),
        "Wo": np.ascontiguousarray(np.asarray(inputs["Wo"], dtype=np.float32)),
    }
    bias_names = tuple(b for b in BIAS_NAMES if np.any(np.asarray(inputs[b])))
    for b in bias_names:
        common[b] = np.ascontiguousarray(
            np.asarray(inputs[b], dtype=np.float32).reshape(1, -1))

    nc = _get_nc(bias_names)
    in_maps = [{"qx": qx[c], "kvx": kvx[c], **common} for c in range(N_CORES)]
    res = run_bass_kernel_spmd(nc, in_maps, core_ids=list(range(N_CORES)),
                               trace=trace, **run_kwargs)

    out = np.empty((B, HH, WW, D), np.float32)
    attn = np.empty((B, NHEADS, S, S), np.float32)
    for c in range(N_CORES):
        r = res.results[c]
        attn[c] = r["attn"].transpose(0, 2, 1)
        out[c] = r["outT"].T.reshape(HH, WW, D)
    return (out, attn), res


def kernel(**inputs):
    (out, attn), _ = _run(inputs)
    return out, attn


# revision 8
# speedup vs baseline: 1.4027x; 1.0858x over previous
"""Trainium2 Bass kernel for nn_AttentionSelfAttention (B=8, S=1024, D=64, 8 heads).

Sharding: data-parallel over batch — one batch element per NeuronCore (8 cores).
Each core runs the full attention block for its batch element; no collectives.

Per-core layout strategy (everything transposed so the softmax statistics fall
out of the matmuls and the big 32MB attention tensor is written at full DMA
bandwidth):
  xqT, xkT   [64, 1024]        (qx+pos)^T, (kvx+pos)^T                  (f32r)
  qT, kT     [128, 4, 1024]    projected q^T / k^T, two heads per 128 rows
  v_aug      [128, h, jc, 65]  v rows with an extra ones column         (f32r)
  scoresT    [128 j, 1024 i] = k_h @ q_h^T        (PSUM, per 128-row j chunk)
  et         [128, 1024]     = exp(scoresT/8)                           (f32r)
  ctx_ps     [65, 512]       = v_aug^T @ et accumulated over j chunks:
                               rows 0-63 = unnormalized ctx^T, row 64 = softmax
                               row-sum (the ones-column trick)
  B          [128, 1024]     = 1/rowsum broadcast over partitions (PE ones-matmul)
  attnT out  [h, j, i]       = et * B   -> host transposes to attn[h, i, j]
  outT out   [64, 1024]      = sum_h Wo_h^T @ (ctxT_h * B) (+ bo), host transposes

Matmuls run in float32r (single-pass reduced-precision fp32, ~1.2e-4 rel err,
4x faster than native fp32 on the PE).
"""

from contextlib import ExitStack

import numpy as np

import concourse.bacc as bacc
import concourse.mybir as mybir
import concourse.tile as tile
from concourse.bass_utils import run_bass_kernel_spmd
from concourse.masks import make_identity

F32 = mybir.dt.float32
F32R = mybir.dt.float32r
EXP = mybir.ActivationFunctionType.Exp

B = 8
HH = 32
WW = 32
D = 64
NH = 4
S = HH * WW
NHEADS = 2 * NH
JC = S // 128
IC = S // 512
N_CORES = 8

BIAS_NAMES = ("bq", "bka", "bva", "bksa", "bvsa", "bo")
NORM_GP_JCS = ()  # which attn-normalize j-chunks run on GpSimd instead of DVE


def declare_io(nc, bias_names=()):
    io = {}
    io["qx"] = nc.dram_tensor("qx", [S, D], F32, kind="ExternalInput").ap()
    io["kvx"] = nc.dram_tensor("kvx", [S, D], F32, kind="ExternalInput").ap()
    io["pos"] = nc.dram_tensor("pos", [S, D], F32, kind="ExternalInput").ap()
    io["Wq"] = nc.dram_tensor("Wq", [D, 512], F32, kind="ExternalInput").ap()
    io["Wka"] = nc.dram_tensor("Wka", [D, 256], F32, kind="ExternalInput").ap()
    io["Wva"] = nc.dram_tensor("Wva", [D, 256], F32, kind="ExternalInput").ap()
    io["Wksa"] = nc.dram_tensor("Wksa", [D, 256], F32, kind="ExternalInput").ap()
    io["Wvsa"] = nc.dram_tensor("Wvsa", [D, 256], F32, kind="ExternalInput").ap()
    io["Wo"] = nc.dram_tensor("Wo", [512, D], F32, kind="ExternalInput").ap()
    for b, n in [("bq", 512), ("bka", 256), ("bva", 256),
                 ("bksa", 256), ("bvsa", 256), ("bo", 64)]:
        if b in bias_names:
            io[b] = nc.dram_tensor(b, [1, n], F32, kind="ExternalInput").ap()
    io["attn"] = nc.dram_tensor("attn", [NHEADS, S, S], F32, kind="ExternalOutput").ap()
    io["outT"] = nc.dram_tensor("outT", [D, S], F32, kind="ExternalOutput").ap()
    return io


def build_attention(ctx: ExitStack, tc, io, bias_names=()):
    nc = tc.nc
    has = lambda b: b in bias_names

    const = ctx.enter_context(tc.tile_pool(name="const", bufs=1))
    wpool = ctx.enter_context(tc.tile_pool(name="weights", bufs=1))
    xpool = ctx.enter_context(tc.tile_pool(name="x", bufs=1))
    qkpool = ctx.enter_context(tc.tile_pool(name="qk", bufs=1))
    etpool = ctx.enter_context(tc.tile_pool(name="et", bufs=8 if bias_names else 9))
    atpool = ctx.enter_context(tc.tile_pool(name="at", bufs=3))
    bpool = ctx.enter_context(tc.tile_pool(name="bcast", bufs=2))
    spool = ctx.enter_context(tc.tile_pool(name="small", bufs=2))
    ctxpool = ctx.enter_context(tc.tile_pool(name="ctxTn", bufs=1))

    ps_s_pool = ctx.enter_context(tc.tile_pool(name="ps_s", bufs=2, space="PSUM"))
    ps_ctx_pool = ctx.enter_context(tc.tile_pool(name="ps_ctx", bufs=2, space="PSUM"))
    ps_misc_pool = ctx.enter_context(tc.tile_pool(name="ps_misc", bufs=2, space="PSUM"))

    # ---- constants ----
    identity = const.tile([128, 128], F32)
    make_identity(nc, identity[:])
    ones_f = const.tile([1, 512], F32)
    nc.vector.memset(ones_f[:], 1.0)
    ones_r = const.tile([1, 512], F32R)
    nc.vector.tensor_copy(ones_r[:], ones_f[:])
    ones_col = const.tile([128, 2], F32)
    nc.vector.memset(ones_col[:], 1.0)
    zero_col = const.tile([128, 2], F32)
    nc.vector.memset(zero_col[:], 0.0)

    BF16 = mybir.dt.bfloat16
    warm_w = const.tile([128, 128], BF16)
    nc.vector.memset(warm_w[:], 1.0)
    warm_x = const.tile([128, 512], BF16)
    nc.vector.memset(warm_x[:], 1.0)
    ps_w = ps_misc_pool.tile([128, 512], F32, tag="mm", name="ps_warm")
    for i in range(16):
        nc.tensor.matmul(ps_w[:], warm_w[:], warm_x[:], start=True, stop=True)

    # ---- weights (DMA with f32->f32r rounding on gpsimd) ----
    wq_t = wpool.tile([D, 512], F32R)
    nc.gpsimd.dma_start(wq_t[:], io["Wq"])
    wka_t = wpool.tile([D, 256], F32R)
    nc.gpsimd.dma_start(wka_t[:], io["Wka"])
    wksa_t = wpool.tile([D, 256], F32R)
    nc.gpsimd.dma_start(wksa_t[:], io["Wksa"])
    wva_t = wpool.tile([D, 256], F32R)
    nc.gpsimd.dma_start(wva_t[:], io["Wva"])
    wvsa_t = wpool.tile([D, 256], F32R)
    nc.gpsimd.dma_start(wvsa_t[:], io["Wvsa"])
    wo_t = wpool.tile([D, NHEADS, D], F32R)
    nc.gpsimd.dma_start(wo_t[:], io["Wo"].rearrange("(h d) e -> d h e", d=D))

    bias_sb = {}
    for b in bias_names:
        n = io[b].shape[1]
        t = wpool.tile([1, n], F32R, name=f"b_{b}")
        nc.gpsimd.dma_start(t[:], io[b])
        bias_sb[b] = t

    def bias_bcast(b, sl, rows, cols):
        """SBUF tile [rows, cols] = bias_sb[b][0, sl] per-partition, bcast over free."""
        ps = ps_misc_pool.tile([rows, cols], F32, tag="mm", name=f"ps_bb_{b}_{sl.start}")
        nc.tensor.matmul(ps[:], bias_sb[b][:, sl], ones_r[:, 0:cols],
                         start=True, stop=True)
        t = wpool.tile([rows, cols], F32, name=f"bb_{b}_{sl.start}")
        nc.scalar.copy(t[:], ps[:])
        return t

    def bias_bcast_free(b, rows):
        """SBUF tile [rows, n] = bias_sb[b] per-free-element, bcast over partitions."""
        n = io[b].shape[1]
        ps = ps_misc_pool.tile([rows, n], F32, tag="mm", name=f"ps_bf_{b}")
        nc.tensor.matmul(ps[:], ones_r[:, 0:rows], bias_sb[b][:],
                         start=True, stop=True)
        t = wpool.tile([rows, n], F32, name=f"bf_{b}")
        nc.scalar.copy(t[:], ps[:])
        return t

    bb_v = {}
    if has("bva"):
        bb_v["bva"] = bias_bcast_free("bva", 128)
    if has("bvsa"):
        bb_v["bvsa"] = bias_bcast_free("bvsa", 128)
    bb_q = [bias_bcast("bq", slice(c * 128, (c + 1) * 128), 128, 512)
            for c in range(4)] if has("bq") else None
    bb_k = []
    for c in range(4):
        bk = "bka" if c < 2 else "bksa"
        cc = c % 2
        bb_k.append(bias_bcast(bk, slice(cc * 128, (cc + 1) * 128), 128, 512)
                    if has(bk) else None)
    bb_o = bias_bcast("bo", slice(0, 64), 64, 512) if has("bo") else None

    # ---- inputs + positional add ----
    xq = xpool.tile([128, JC, D], F32)
    nc.sync.dma_start(xq[:], io["qx"].rearrange("(o p) d -> p o d", p=128))
    xk = xpool.tile([128, JC, D], F32)
    nc.sync.dma_start(xk[:], io["kvx"].rearrange("(o p) d -> p o d", p=128))
    pos_t = xpool.tile([128, JC, D], F32)
    nc.sync.dma_start(pos_t[:], io["pos"].rearrange("(o p) d -> p o d", p=128))
    nc.vector.tensor_add(xq[:], xq[:], pos_t[:])
    nc.vector.tensor_add(xk[:], xk[:], pos_t[:])

    # ---- transpose inputs: xT[d, i] (f32r) ----
    xqT = xpool.tile([D, S], F32R)
    xkT = xpool.tile([D, S], F32R)
    for xt, x in ((xqT, xq), (xkT, xk)):
        for oc in range(JC):
            ps_t = ps_misc_pool.tile([D, 128], F32, tag="mm")
            nc.tensor.transpose(ps_t[:], x[:, oc, :], identity[:])
            nc.vector.tensor_copy(xt[:, oc * 128:(oc + 1) * 128], ps_t[:])

    # ---- v projection (+ ones column) ----
    # v_aug[p, h, jc, 0:64] = (x_p @ Wv + bv)[jc*128+p, h*64:...]; [..., 64] = 1
    v_aug = xpool.tile([128, NHEADS, JC, 65], F32R)
    for g, (xt, wv, bv) in enumerate(
        ((xkT, wva_t, "bva"), (xqT, wvsa_t, "bvsa"))
    ):
        for jc in range(JC):
            ps_v = ps_misc_pool.tile([128, 256], F32, tag="mm")
            nc.tensor.matmul(ps_v[:], xt[:, jc * 128:(jc + 1) * 128], wv[:],
                             start=True, stop=True)
            dst = v_aug[:, g * 4:(g + 1) * 4, jc, 0:D]
            if has(bv):
                nc.vector.tensor_add(
                    dst, ps_v[:].rearrange("p (h d) -> p h d", h=4),
                    bb_v[bv][:].rearrange("p (h d) -> p h d", h=4))
            else:
                nc.vector.tensor_copy(dst, ps_v[:].rearrange("p (h d) -> p h d", h=4))
    nc.vector.tensor_copy(
        v_aug[:, :, :, D:D + 1],
        ones_col[:, 0:1].unsqueeze(1).to_broadcast([128, NHEADS, JC, 1]))

    # ---- q/k head projections, 2 heads (128 rows) per matmul ----
    # qT_all[p, c, i]: rows p = (h%2)*64+d for head h = 2c + p//64
    qT_all = qkpool.tile([128, 4, S], F32R, tag="qT")
    for ic in range(IC):
        for c in range(4):
            ps_q = ps_misc_pool.tile([128, 512], F32, tag="mm")
            nc.tensor.matmul(ps_q[:], wq_t[:, c * 128:(c + 1) * 128],
                             xqT[:, ic * 512:(ic + 1) * 512],
                             start=True, stop=True)
            dst = qT_all[:, c, ic * 512:(ic + 1) * 512]
            if has("bq"):
                nc.vector.tensor_add(dst, ps_q[:], bb_q[c][:])
            else:
                nc.vector.tensor_copy(dst, ps_q[:])

    # kT: heads 0-3 = Wka on xkT; heads 4-7 = Wksa on xqT.
    # Stored zero-padded to the full 128 contraction rows per head so every
    # scores matmul is K=128, uniform with the ctx matmuls (K-switching
    # between matmuls costs ~165ns each on the fp32r path).
    kT_z = qkpool.tile([128, NHEADS, S], F32R, tag="kT")
    for h in range(NHEADS):
        po = 64 - (h % 2) * 64  # the parity rows this head does NOT occupy
        nc.vector.tensor_copy(
            kT_z[po:po + 64, h, :],
            zero_col[po:po + 64, 0:1].to_broadcast([64, S]))
    for ic in range(IC):
        for c in range(4):
            xt = xkT if c < 2 else xqT
            wk = wka_t if c < 2 else wksa_t
            bk = "bka" if c < 2 else "bksa"
            cc = c % 2
            ps_k = ps_misc_pool.tile([128, 512], F32, tag="mm")
            nc.tensor.matmul(ps_k[:], wk[:, cc * 128:(cc + 1) * 128],
                             xt[:, ic * 512:(ic + 1) * 512],
                             start=True, stop=True)
            for par in range(2):
                h = 2 * c + par
                dst = kT_z[par * 64:par * 64 + 64, h, ic * 512:(ic + 1) * 512]
                ps_sl = ps_k[par * 64:par * 64 + 64, :]
                if has(bk):
                    nc.vector.tensor_add(dst, ps_sl, bb_k[c][par * 64:par * 64 + 64, :])
                else:
                    nc.vector.tensor_copy(dst, ps_sl)

    def qT_h(h):
        return qT_all[(h % 2) * 64:(h % 2) * 64 + 64, h // 2, :]

    # ---- per-head attention ----
    ps_w2 = ps_s_pool.tile([128, S], F32, tag="s", name="ps_warm2")
    for i in range(8):
        nc.tensor.matmul(ps_w2[:, 0:512], warm_w[:], warm_x[:], start=True, stop=True)

    ctxTn = ctxpool.tile([D, NHEADS, S], F32R)
    for h in range(NHEADS):
        ps_c = [ps_ctx_pool.tile([65, 512], F32, tag="ctx", name=f"ps_c{h}_{i}")
                for i in range(IC)]
        ets = []
        pend = None
        for jc in range(JC):
            ps_s = ps_s_pool.tile([128, S], F32, tag="s")
            # software pipeline: scores(jc) interleaved with ctx(jc-1) so no
            # two consecutive PE matmuls share a stationary operand.
            kTj = kT_z[:, h, jc * 128:(jc + 1) * 128]
            qTc = qT_all[:, h // 2, :]
            nc.tensor.matmul(ps_s[:, 0:512], kTj,
                             qTc[:, 0:512], start=True, stop=True)
            if pend is not None:
                pet, pjc = pend
                nc.tensor.matmul(ps_c[0][:], v_aug[:, h, pjc, :],
                                 pet[:, 0:512], start=(pjc == 0), stop=False)
            nc.tensor.matmul(ps_s[:, 512:1024], kTj,
                             qTc[:, 512:1024], start=True, stop=True)
            if pend is not None:
                pet, pjc = pend
                nc.tensor.matmul(ps_c[1][:], v_aug[:, h, pjc, :],
                                 pet[:, 512:1024], start=(pjc == 0), stop=False)
            et = etpool.tile([128, S], F32R, tag="et")
            nc.scalar.activation(et[:], ps_s[:], EXP, scale=0.125)
            ets.append(et)
            pend = (et, jc)
        # epilogue ctx pair
        nc.tensor.matmul(ps_c[0][:], v_aug[:, h, JC - 1, :],
                         ets[JC - 1][:, 0:512], start=False, stop=True)
        nc.tensor.matmul(ps_c[1][:], v_aug[:, h, JC - 1, :],
                         ets[JC - 1][:, 512:1024], start=False, stop=True)

        # 1/rowsum: spread the 1024 sums over 128 partitions by DMA (cheap
        # elementwise reciprocal), gather back, broadcast over partitions.
        sums = spool.tile([1, S], F32, tag="sums")
        nc.scalar.copy(sums[:, 0:512], ps_c[0][64:65, :])
        nc.scalar.copy(sums[:, 512:1024], ps_c[1][64:65, :])
        sp = spool.tile([128, 8], F32, tag="sp")
        nc.sync.dma_start(sp[:], sums[:])
        nc.vector.reciprocal(sp[:], sp[:])
        rrow = spool.tile([1, S], F32, tag="rrow")
        nc.sync.dma_start(rrow[:], sp[:])
        B_t = bpool.tile([128, S], F32, tag="B")
        nc.gpsimd.partition_broadcast(B_t[:, 0:512], rrow[:, 0:512])
        nc.gpsimd.partition_broadcast(B_t[:, 512:1024], rrow[:, 512:1024])
        for ic in range(IC):
            nc.vector.tensor_mul(ctxTn[:, h, ic * 512:(ic + 1) * 512],
                                 ps_c[ic][0:D, :],
                                 B_t[0:D, ic * 512:(ic + 1) * 512])
        # normalize attn^T and write out (NORM_GP_JCS tiles go to GpSimd)
        for jc in range(JC):
            at = atpool.tile([128, S], F32, tag="at")
            eng = nc.gpsimd if jc in NORM_GP_JCS else nc.vector
            eng.tensor_mul(at[:], ets[jc][:].bitcast(F32), B_t[:])
            nc.sync.dma_start(io["attn"][h, jc * 128:(jc + 1) * 128, :], at[:])

    # ---- output projection: outT = sum_h Wo_h^T @ ctxTn_h (+ bo) ----
    for ic in range(IC):
        ps_o = ps_misc_pool.tile([D, 512], F32, tag="mm")
        for h in range(NHEADS):
            nc.tensor.matmul(ps_o[:], wo_t[:, h, :],
                             ctxTn[:, h, ic * 512:(ic + 1) * 512],
                             start=(h == 0), stop=(h == NHEADS - 1))
        ot = spool.tile([D, 512], F32, tag="ot")
        if has("bo"):
            nc.vector.tensor_add(ot[:], ps_o[:], bb_o[:])
        else:
            nc.vector.tensor_copy(ot[:], ps_o[:])
        nc.sync.dma_start(io["outT"][:, ic * 512:(ic + 1) * 512], ot[:])


_nc_cache = {}


def _get_nc(bias_names):
    key = tuple(bias_names)
    if key not in _nc_cache:
        nc = bacc.Bacc()
        io = declare_io(nc, bias_names)
        with tile.TileContext(nc) as tc:
            with ExitStack() as ctx:
                build_attention(ctx, tc, io, bias_names)
        nc.finalize()
        _nc_cache[key] = nc
    return _nc_cache[key]


def _run(inputs, trace=False, **run_kwargs):
    qx = np.ascontiguousarray(np.asarray(inputs["qx"], dtype=np.float32).reshape(B, S, D))
    kvx = np.ascontiguousarray(np.asarray(inputs["kvx"], dtype=np.float32).reshape(B, S, D))
    pos = np.ascontiguousarray(np.asarray(inputs["pos_table"], dtype=np.float32))
    common = {
        "pos": pos,
        "Wq": np.ascontiguousarray(np.asarray(inputs["Wq"], dtype=np.float32)),
        "Wka": np.ascontiguousarray(np.asarray(inputs["Wka"], dtype=np.float32)),
        "Wva": np.ascontiguousarray(np.asarray(inputs["Wva"], dtype=np.float32)),
        "Wksa": np.ascontiguousarray(np.asarray(inputs["Wksa"], dtype=np.float32)),
        "Wvsa": np.ascontiguousarray(np.asarray(inputs["Wvsa"], dtype=np.float32)),
        "Wo": np.ascontiguousarray(np.asarray(inputs["Wo"], dtype=np.float32)),
    }
    bias_names = tuple(b for b in BIAS_NAMES if np.any(np.asarray(inputs[b])))
    for b in bias_names:
        common[b] = np.ascontiguousarray(
            np.asarray(inputs[b], dtype=np.float32).reshape(1, -1))

    nc = _get_nc(bias_names)
    in_maps = [{"qx": qx[c], "kvx": kvx[c], **common} for c in range(N_CORES)]
    res = run_bass_kernel_spmd(nc, in_maps, core_ids=list(range(N_CORES)),
                               trace=trace, **run_kwargs)

    out = np.empty((B, HH, WW, D), np.float32)
    attn = np.empty((B, NHEADS, S, S), np.float32)
    for c in range(N_CORES):
        r = res.results[c]
        attn[c] = r["attn"].transpose(0, 2, 1)
        out[c] = r["outT"].T.reshape(HH, WW, D)
    return (out, attn), res


def kernel(**inputs):
    (out, attn), _ = _run(inputs)
    return out, attn


# revision 9
# speedup vs baseline: 2.0935x; 1.4925x over previous
"""Trainium2 Bass kernel for nn_AttentionSelfAttention (B=8, S=1024, D=64, 8 heads).

Sharding: data-parallel over batch — one batch element per NeuronCore (8 cores).
Each core runs the full attention block for its batch element; no collectives.

Per-core layout strategy (everything transposed so the softmax statistics fall
out of the matmuls and the big 32MB attention tensor is written at full DMA
bandwidth):
  xqT, xkT   [64, 1024]        (qx+pos)^T, (kvx+pos)^T                  (f32r)
  qT, kT     [128, 4, 1024]    projected q^T / k^T, two heads per 128 rows
  v_aug      [128, h, jc, 65]  v rows with an extra ones column         (f32r)
  scoresT    [128 j, 1024 i] = k_h @ q_h^T        (PSUM, per 128-row j chunk)
  et         [128, 1024]     = exp(scoresT/8)                           (f32r)
  ctx_ps     [65, 512]       = v_aug^T @ et accumulated over j chunks:
                               rows 0-63 = unnormalized ctx^T, row 64 = softmax
                               row-sum (the ones-column trick)
  B          [128, 1024]     = 1/rowsum broadcast over partitions (PE ones-matmul)
  attnT out  [h, j, i]       = et * B   -> host transposes to attn[h, i, j]
  outT out   [64, 1024]      = sum_h Wo_h^T @ (ctxT_h * B) (+ bo), host transposes

Matmuls run in float32r (single-pass reduced-precision fp32, ~1.2e-4 rel err,
4x faster than native fp32 on the PE).
"""

import os
from contextlib import ExitStack

import numpy as np

import concourse.bacc as bacc
import concourse.mybir as mybir
import concourse.tile as tile
from concourse.bass_utils import run_bass_kernel_spmd
from concourse.masks import make_identity

F32 = mybir.dt.float32
F32R = mybir.dt.float32r
EXP = mybir.ActivationFunctionType.Exp

B = 8
HH = 32
WW = 32
D = 64
NH = 4
S = HH * WW
NHEADS = 2 * NH
JC = S // 128
IC = S // 512
N_CORES = 8

BIAS_NAMES = ("bq", "bka", "bva", "bksa", "bvsa", "bo")
NORM_GP_JCS = ()  # which attn-normalize j-chunks run on GpSimd instead of DVE


def declare_io(nc, bias_names=()):
    io = {}
    io["qx"] = nc.dram_tensor("qx", [S, D], F32, kind="ExternalInput").ap()
    io["kvx"] = nc.dram_tensor("kvx", [S, D], F32, kind="ExternalInput").ap()
    io["pos"] = nc.dram_tensor("pos", [S, D], F32, kind="ExternalInput").ap()
    io["Wq"] = nc.dram_tensor("Wq", [D, 512], F32, kind="ExternalInput").ap()
    io["Wka"] = nc.dram_tensor("Wka", [D, 256], F32, kind="ExternalInput").ap()
    io["Wva"] = nc.dram_tensor("Wva", [D, 256], F32, kind="ExternalInput").ap()
    io["Wksa"] = nc.dram_tensor("Wksa", [D, 256], F32, kind="ExternalInput").ap()
    io["Wvsa"] = nc.dram_tensor("Wvsa", [D, 256], F32, kind="ExternalInput").ap()
    io["Wo"] = nc.dram_tensor("Wo", [512, D], F32, kind="ExternalInput").ap()
    for b, n in [("bq", 512), ("bka", 256), ("bva", 256),
                 ("bksa", 256), ("bvsa", 256), ("bo", 64)]:
        if b in bias_names:
            io[b] = nc.dram_tensor(b, [1, n], F32, kind="ExternalInput").ap()
    io["attn"] = nc.dram_tensor("attn", [NHEADS, S, S], F32, kind="ExternalOutput").ap()
    io["outT"] = nc.dram_tensor("outT", [D, S], F32, kind="ExternalOutput").ap()
    return io


def build_attention(ctx: ExitStack, tc, io, bias_names=()):
    nc = tc.nc
    has = lambda b: b in bias_names

    const = ctx.enter_context(tc.tile_pool(name="const", bufs=1))
    wpool = ctx.enter_context(tc.tile_pool(name="weights", bufs=1))
    xpool = ctx.enter_context(tc.tile_pool(name="x", bufs=1))
    qkpool = ctx.enter_context(tc.tile_pool(name="qk", bufs=1))
    etpool = ctx.enter_context(tc.tile_pool(name="et", bufs=8 if bias_names else 9))
    atpool = ctx.enter_context(tc.tile_pool(name="at", bufs=3))
    bpool = ctx.enter_context(tc.tile_pool(name="bcast", bufs=2))
    spool = ctx.enter_context(tc.tile_pool(name="small", bufs=2))
    ctxpool = ctx.enter_context(tc.tile_pool(name="ctxTn", bufs=1))

    ps_s_pool = ctx.enter_context(tc.tile_pool(name="ps_s", bufs=2, space="PSUM"))
    ps_ctx_pool = ctx.enter_context(tc.tile_pool(name="ps_ctx", bufs=2, space="PSUM"))
    ps_misc_pool = ctx.enter_context(tc.tile_pool(name="ps_misc", bufs=2, space="PSUM"))

    # ---- constants ----
    identity = const.tile([128, 128], F32)
    make_identity(nc, identity[:])
    ones_f = const.tile([1, 512], F32)
    nc.vector.memset(ones_f[:], 1.0)
    ones_r = const.tile([1, 512], F32R)
    nc.vector.tensor_copy(ones_r[:], ones_f[:])
    ones_col = const.tile([128, 2], F32)
    nc.vector.memset(ones_col[:], 1.0)
    zero_col = const.tile([128, 2], F32)
    nc.vector.memset(zero_col[:], 0.0)

    BF16 = mybir.dt.bfloat16
    warm_w = const.tile([128, 128], BF16)
    nc.vector.memset(warm_w[:], 1.0)
    warm_x = const.tile([128, 512], BF16)
    nc.vector.memset(warm_x[:], 1.0)
    ps_w = ps_misc_pool.tile([128, 512], F32, tag="mm", name="ps_warm")
    for i in range(16):
        nc.tensor.matmul(ps_w[:], warm_w[:], warm_x[:], start=True, stop=True)

    # ---- weights (DMA with f32->f32r rounding on gpsimd) ----
    wq_t = wpool.tile([D, 512], F32R)
    nc.gpsimd.dma_start(wq_t[:], io["Wq"])
    wka_t = wpool.tile([D, 256], F32R)
    nc.gpsimd.dma_start(wka_t[:], io["Wka"])
    wksa_t = wpool.tile([D, 256], F32R)
    nc.gpsimd.dma_start(wksa_t[:], io["Wksa"])
    wva_t = wpool.tile([D, 256], F32R)
    nc.gpsimd.dma_start(wva_t[:], io["Wva"])
    wvsa_t = wpool.tile([D, 256], F32R)
    nc.gpsimd.dma_start(wvsa_t[:], io["Wvsa"])
    wo_t = wpool.tile([D, NHEADS, D], F32R)
    nc.gpsimd.dma_start(wo_t[:], io["Wo"].rearrange("(h d) e -> d h e", d=D))

    bias_sb = {}
    for b in bias_names:
        n = io[b].shape[1]
        t = wpool.tile([1, n], F32R, name=f"b_{b}")
        nc.gpsimd.dma_start(t[:], io[b])
        bias_sb[b] = t

    def bias_bcast(b, sl, rows, cols):
        """SBUF tile [rows, cols] = bias_sb[b][0, sl] per-partition, bcast over free."""
        ps = ps_misc_pool.tile([rows, cols], F32, tag="mm", name=f"ps_bb_{b}_{sl.start}")
        nc.tensor.matmul(ps[:], bias_sb[b][:, sl], ones_r[:, 0:cols],
                         start=True, stop=True)
        t = wpool.tile([rows, cols], F32, name=f"bb_{b}_{sl.start}")
        nc.scalar.copy(t[:], ps[:])
        return t

    def bias_bcast_free(b, rows):
        """SBUF tile [rows, n] = bias_sb[b] per-free-element, bcast over partitions."""
        n = io[b].shape[1]
        ps = ps_misc_pool.tile([rows, n], F32, tag="mm", name=f"ps_bf_{b}")
        nc.tensor.matmul(ps[:], ones_r[:, 0:rows], bias_sb[b][:],
                         start=True, stop=True)
        t = wpool.tile([rows, n], F32, name=f"bf_{b}")
        nc.scalar.copy(t[:], ps[:])
        return t

    bb_v = {}
    if has("bva"):
        bb_v["bva"] = bias_bcast_free("bva", 128)
    if has("bvsa"):
        bb_v["bvsa"] = bias_bcast_free("bvsa", 128)
    bb_q = [bias_bcast("bq", slice(c * 128, (c + 1) * 128), 128, 512)
            for c in range(4)] if has("bq") else None
    bb_k = []
    for c in range(4):
        bk = "bka" if c < 2 else "bksa"
        cc = c % 2
        bb_k.append(bias_bcast(bk, slice(cc * 128, (cc + 1) * 128), 128, 512)
                    if has(bk) else None)
    bb_o = bias_bcast("bo", slice(0, 64), 64, 512) if has("bo") else None

    # ---- inputs + positional add ----
    xq = xpool.tile([128, JC, D], F32)
    nc.sync.dma_start(xq[:], io["qx"].rearrange("(o p) d -> p o d", p=128))
    xk = xpool.tile([128, JC, D], F32)
    nc.sync.dma_start(xk[:], io["kvx"].rearrange("(o p) d -> p o d", p=128))
    pos_t = xpool.tile([128, JC, D], F32)
    nc.sync.dma_start(pos_t[:], io["pos"].rearrange("(o p) d -> p o d", p=128))
    nc.vector.tensor_add(xq[:], xq[:], pos_t[:])
    nc.vector.tensor_add(xk[:], xk[:], pos_t[:])

    # ---- transpose inputs: xT[d, i] (f32r) ----
    xqT = xpool.tile([D, S], F32R)
    xkT = xpool.tile([D, S], F32R)
    for xt, x in ((xqT, xq), (xkT, xk)):
        for oc in range(JC):
            ps_t = ps_misc_pool.tile([D, 128], F32, tag="mm")
            nc.tensor.transpose(ps_t[:], x[:, oc, :], identity[:])
            nc.vector.tensor_copy(xt[:, oc * 128:(oc + 1) * 128], ps_t[:])

    # ---- v projection (+ ones column) ----
    # v_aug[p, h, jc, 0:64] = (x_p @ Wv + bv)[jc*128+p, h*64:...]; [..., 64] = 1
    v_aug = xpool.tile([128, NHEADS, JC, 65], F32R)
    for g, (xt, wv, bv) in enumerate(
        ((xkT, wva_t, "bva"), (xqT, wvsa_t, "bvsa"))
    ):
        for jc in range(JC):
            ps_v = ps_misc_pool.tile([128, 256], F32, tag="mm")
            nc.tensor.matmul(ps_v[:], xt[:, jc * 128:(jc + 1) * 128], wv[:],
                             start=True, stop=True)
            dst = v_aug[:, g * 4:(g + 1) * 4, jc, 0:D]
            if has(bv):
                nc.vector.tensor_add(
                    dst, ps_v[:].rearrange("p (h d) -> p h d", h=4),
                    bb_v[bv][:].rearrange("p (h d) -> p h d", h=4))
            else:
                nc.vector.tensor_copy(dst, ps_v[:].rearrange("p (h d) -> p h d", h=4))
    nc.vector.tensor_copy(
        v_aug[:, :, :, D:D + 1],
        ones_col[:, 0:1].unsqueeze(1).to_broadcast([128, NHEADS, JC, 1]))

    # ---- q/k head projections, 2 heads (128 rows) per matmul ----
    # qT_all[p, c, i]: rows p = (h%2)*64+d for head h = 2c + p//64
    qT_all = qkpool.tile([128, 4, S], F32R, tag="qT")
    for ic in range(IC):
        for c in range(4):
            ps_q = ps_misc_pool.tile([128, 512], F32, tag="mm")
            nc.tensor.matmul(ps_q[:], wq_t[:, c * 128:(c + 1) * 128],
                             xqT[:, ic * 512:(ic + 1) * 512],
                             start=True, stop=True)
            dst = qT_all[:, c, ic * 512:(ic + 1) * 512]
            if has("bq"):
                nc.vector.tensor_add(dst, ps_q[:], bb_q[c][:])
            else:
                nc.vector.tensor_copy(dst, ps_q[:])

    # kT: heads 0-3 = Wka on xkT; heads 4-7 = Wksa on xqT.
    # Stored zero-padded to the full 128 contraction rows per head so every
    # scores matmul is K=128, uniform with the ctx matmuls (K-switching
    # between matmuls costs ~165ns each on the fp32r path).
    kT_z = qkpool.tile([128, NHEADS, S], F32R, tag="kT")
    for h in range(NHEADS):
        po = 64 - (h % 2) * 64  # the parity rows this head does NOT occupy
        nc.vector.tensor_copy(
            kT_z[po:po + 64, h, :],
            zero_col[po:po + 64, 0:1].to_broadcast([64, S]))
    for ic in range(IC):
        for c in range(4):
            xt = xkT if c < 2 else xqT
            wk = wka_t if c < 2 else wksa_t
            bk = "bka" if c < 2 else "bksa"
            cc = c % 2
            ps_k = ps_misc_pool.tile([128, 512], F32, tag="mm")
            nc.tensor.matmul(ps_k[:], wk[:, cc * 128:(cc + 1) * 128],
                             xt[:, ic * 512:(ic + 1) * 512],
                             start=True, stop=True)
            for par in range(2):
                h = 2 * c + par
                dst = kT_z[par * 64:par * 64 + 64, h, ic * 512:(ic + 1) * 512]
                ps_sl = ps_k[par * 64:par * 64 + 64, :]
                if has(bk):
                    nc.vector.tensor_add(dst, ps_sl, bb_k[c][par * 64:par * 64 + 64, :])
                else:
                    nc.vector.tensor_copy(dst, ps_sl)

    def qT_h(h):
        return qT_all[(h % 2) * 64:(h % 2) * 64 + 64, h // 2, :]

    # ---- per-head attention ----
    ps_w2 = ps_s_pool.tile([128, S], F32, tag="s", name="ps_warm2")
    for i in range(8):
        nc.tensor.matmul(ps_w2[:, 0:512], warm_w[:], warm_x[:], start=True, stop=True)

    ctxTn = ctxpool.tile([D, NHEADS, S], F32R)
    for h in range(NHEADS):
        ps_c = [ps_ctx_pool.tile([65, 512], F32, tag="ctx", name=f"ps_c{h}_{i}")
                for i in range(IC)]
        ets = []
        pend = None
        for jc in range(JC):
            ps_s = ps_s_pool.tile([128, S], F32, tag="s")
            # software pipeline: scores(jc) interleaved with ctx(jc-1) so no
            # two consecutive PE matmuls share a stationary operand.
            kTj = kT_z[:, h, jc * 128:(jc + 1) * 128]
            qTc = qT_all[:, h // 2, :]
            nc.tensor.matmul(ps_s[:, 0:512], kTj,
                             qTc[:, 0:512], start=True, stop=True)
            if pend is not None:
                pet, pjc = pend
                nc.tensor.matmul(ps_c[0][:], v_aug[:, h, pjc, :],
                                 pet[:, 0:512], start=(pjc == 0), stop=False)
            nc.tensor.matmul(ps_s[:, 512:1024], kTj,
                             qTc[:, 512:1024], start=True, stop=True)
            if pend is not None:
                pet, pjc = pend
                nc.tensor.matmul(ps_c[1][:], v_aug[:, h, pjc, :],
                                 pet[:, 512:1024], start=(pjc == 0), stop=False)
            et = etpool.tile([128, S], F32R, tag="et")
            nc.scalar.activation(et[:], ps_s[:], EXP, scale=0.125)
            ets.append(et)
            pend = (et, jc)
        # epilogue ctx pair
        nc.tensor.matmul(ps_c[0][:], v_aug[:, h, JC - 1, :],
                         ets[JC - 1][:, 0:512], start=False, stop=True)
        nc.tensor.matmul(ps_c[1][:], v_aug[:, h, JC - 1, :],
                         ets[JC - 1][:, 512:1024], start=False, stop=True)

        # 1/rowsum: spread the 1024 sums over 128 partitions by DMA (cheap
        # elementwise reciprocal), gather back, broadcast over partitions.
        sums = spool.tile([1, S], F32, tag="sums")
        nc.scalar.copy(sums[:, 0:512], ps_c[0][64:65, :])
        nc.scalar.copy(sums[:, 512:1024], ps_c[1][64:65, :])
        sp = spool.tile([128, 8], F32, tag="sp")
        nc.sync.dma_start(sp[:], sums[:])
        nc.vector.reciprocal(sp[:], sp[:])
        rrow = spool.tile([1, S], F32, tag="rrow")
        nc.sync.dma_start(rrow[:], sp[:])
        B_t = bpool.tile([128, S], F32, tag="B")
        nc.gpsimd.partition_broadcast(B_t[:, 0:512], rrow[:, 0:512])
        nc.gpsimd.partition_broadcast(B_t[:, 512:1024], rrow[:, 512:1024])
        for ic in range(IC):
            nc.vector.tensor_mul(ctxTn[:, h, ic * 512:(ic + 1) * 512],
                                 ps_c[ic][0:D, :],
                                 B_t[0:D, ic * 512:(ic + 1) * 512])
        # normalize attn^T and write out (NORM_GP_JCS tiles go to GpSimd)
        for jc in range(JC):
            if os.environ.get("ATTN_SKIP_NORM"):
                break
            at = atpool.tile([128, S], F32, tag="at")
            eng = nc.gpsimd if jc in NORM_GP_JCS else nc.vector
            eng.tensor_mul(at[:], ets[jc][:].bitcast(F32), B_t[:])
            if not os.environ.get("ATTN_SKIP_DMA"):
                nc.sync.dma_start(io["attn"][h, jc * 128:(jc + 1) * 128, :], at[:])

    # ---- output projection: outT = sum_h Wo_h^T @ ctxTn_h (+ bo) ----
    for ic in range(IC):
        ps_o = ps_misc_pool.tile([D, 512], F32, tag="mm")
        for h in range(NHEADS):
            nc.tensor.matmul(ps_o[:], wo_t[:, h, :],
                             ctxTn[:, h, ic * 512:(ic + 1) * 512],
                             start=(h == 0), stop=(h == NHEADS - 1))
        ot = spool.tile([D, 512], F32, tag="ot")
        if has("bo"):
            nc.vector.tensor_add(ot[:], ps_o[:], bb_o[:])
        else:
            nc.vector.tensor_copy(ot[:], ps_o[:])
        nc.sync.dma_start(io["outT"][:, ic * 512:(ic + 1) * 512], ot[:])


_nc_cache = {}


def _get_nc(bias_names):
    key = tuple(bias_names)
    if key not in _nc_cache:
        nc = bacc.Bacc()
        io = declare_io(nc, bias_names)
        with tile.TileContext(nc) as tc:
            with ExitStack() as ctx:
                build_attention(ctx, tc, io, bias_names)
        nc.finalize()
        _nc_cache[key] = nc
    return _nc_cache[key]


def _run(inputs, trace=False, **run_kwargs):
    qx = np.ascontiguousarray(np.asarray(inputs["qx"], dtype=np.float32).reshape(B, S, D))
    kvx = np.ascontiguousarray(np.asarray(inputs["kvx"], dtype=np.float32).reshape(B, S, D))
    pos = np.ascontiguousarray(np.asarray(inputs["pos_table"], dtype=np.float32))
    common = {
        "pos": pos,
        "Wq": np.ascontiguousarray(np.asarray(inputs["Wq"], dtype=np.float32)),
        "Wka": np.ascontiguousarray(np.asarray(inputs["Wka"], dtype=np.float32)),
        "Wva": np.ascontiguousarray(np.asarray(inputs["Wva"], dtype=np.float32)),
        "Wksa": np.ascontiguousarray(np.asarray(inputs["Wksa"], dtype=np.float32)),
        "Wvsa": np.ascontiguousarray(np.asarray(inputs["Wvsa"], dtype=np.float32)),
        "Wo": np.ascontiguousarray(np.asarray(inputs["Wo"], dtype=np.float32)),
    }
    bias_names = tuple(b for b in BIAS_NAMES if np.any(np.asarray(inputs[b])))
    for b in bias_names:
        common[b] = np.ascontiguousarray(
            np.asarray(inputs[b], dtype=np.float32).reshape(1, -1))

    nc = _get_nc(bias_names)
    in_maps = [{"qx": qx[c], "kvx": kvx[c], **common} for c in range(N_CORES)]
    res = run_bass_kernel_spmd(nc, in_maps, core_ids=list(range(N_CORES)),
                               trace=trace, **run_kwargs)

    out = np.empty((B, HH, WW, D), np.float32)
    attn = np.empty((B, NHEADS, S, S), np.float32)
    for c in range(N_CORES):
        r = res.results[c]
        attn[c] = r["attn"].transpose(0, 2, 1)
        out[c] = r["outT"].T.reshape(HH, WW, D)
    return (out, attn), res


def kernel(**inputs):
    (out, attn), _ = _run(inputs)
    return out, attn
